# revision 20
# baseline (speedup 1.0000x reference)
"""Trainium2 Bass kernel for nn_DiseaseModel_mlp (GNN message passing + MLP decoder).

Data parallel over batch: 64 graphs -> 8 NeuronCores x 8 graphs; weights
replicated. Per core the 8 graphs are split into 2 groups of 4 that are
pipelined across engines (PE / ACT / DVE / Pool).

Key structure (fp16 on device, fp32 PSUM accumulation):
- state kept feature-major [35, G, 128] (row 34 = ones -> bias folding)
- per round: ht = relu(Wg^T @ state) (feature-major), h = relu(state^T Wg)
  (node-major), f = a12aug^T @ [ht;1] gives rows [f_dst, ones, f_src] in
  one matmul; e[q,p] = f_dst[q]+f_src[p] via one rank-2 matmul per graph;
  lrelu via ACT Prelu(alpha=.01) in-place on PSUM; mask folded as -25 on
  masked entries via an fp8 identity x moff matmul accumulated onto the
  same PSUM; exp on ACT; U = P^T @ [h|1] gives messages + softmax
  row-sums in one matmul. No DVE mask multiply, no max-subtraction.
- decoder cell_emb branch (4/5 of W1) precomputed during round 0.

Note: every matmul operand is kept at SBUF base partition 0 (operands at
other bases can hard-crash the device).
"""

import numpy as np

ATOM = 34
HID = 256
LATENT = 128
CELLS = 512
B, N = 64, 128
NCORES = 8
G = B // NCORES          # graphs per core = 8
NG = 4                   # graphs per pipeline group
MOFF = -25.0             # additive mask offset (post-lrelu), exp(-25+6)~0
N_WARM_MM = 12           # dummy matmuls to ramp the PE p-state during DMA

_CACHE = {}


def _build_bass(dbg=0):
    import concourse.bass as bass
    import concourse.bacc as bacc
    import concourse.mybir as mybir
    import concourse.tile as tile

    f32 = mybir.dt.float32
    f16 = mybir.dt.float16
    f8 = mybir.dt.float8e4
    AF = mybir.ActivationFunctionType
    OP = mybir.AluOpType
    AX = mybir.AxisListType

    nc = bacc.Bacc(None)

    # wA: wg(3x34) wg1(3x34) wg2(3x34) s1c(3x128) s2c(3x128) i128
    FA = 9 * ATOM + 6 * 128 + 128
    FB = 256 + 68 + 128 + 640 + 256 + 1024 + 4

    d_x0t = nc.declare_dram_parameter("x0t", [ATOM + 1, G, N], f16, isOutput=False)
    d_m01 = nc.declare_dram_parameter("m01", [128, G * N], f16, isOutput=False)
    d_cellT = nc.declare_dram_parameter("cellT", [128, 4, G], f16, isOutput=False)
    d_wA = nc.declare_dram_parameter("wA", [128, FA], f16, isOutput=False)
    d_wB = nc.declare_dram_parameter("wB", [128, FB], f16, isOutput=False)
    d_b32 = nc.declare_dram_parameter("b32", [128, 9], f32, isOutput=False)
    d_score = nc.declare_dram_parameter("score", [G, 1], f32, isOutput=True)

    _dbgshape = {1: [ATOM, G, N], 2: [ATOM, G, N], 3: [ATOM, G, N],
                 4: [ATOM, G, N],  5: [128, G, N], 6: [ATOM, G, N],
                 7: [128, G, ATOM + 1], 8: [128, G, ATOM], 9: [128, 2, G, N],
                 10: [ATOM, G, N], 11: [128, G], 12: [128, G],
                 13: [128, G], 14: [128, 2, G], 15: [1, G]}.get(dbg, [1, 1])
    d_xdbg = (nc.declare_dram_parameter("xdbg", _dbgshape, f32, isOutput=True)
              if dbg else None)

    groups = [(0, slice(0, NG)), (1, slice(NG, G))]

    lowp = nc.allow_low_precision(reason="fp16 pipeline; tolerance 2e-2")
    lowp.__enter__()
    with tile.TileContext(nc) as tc:
        with (
            tc.tile_pool(name="singles", bufs=1) as singles,
            tc.tile_pool(name="work", bufs=2) as work,
            tc.tile_pool(name="pbig", bufs=1, space="PSUM") as pbig,
            tc.tile_pool(name="patt", bufs=1, space="PSUM") as patt,
            tc.tile_pool(name="pmisc", bufs=1, space="PSUM") as pmisc,
            tc.tile_pool(name="pdT", bufs=1, space="PSUM") as pdT,
        ):
            # ---------------- input DMAs (issue ASAP, big ones first) ------
            wA_sb = singles.tile([128, FA], f16, tag="wA")
            nc.scalar.dma_start(out=wA_sb, in_=d_wA[:])
            state0 = singles.tile([ATOM + 1, G, N], f16, tag="state0")
            nc.sync.dma_start(out=state0[:, 0:NG, :], in_=d_x0t[:, 0:NG, :])
            nc.sync.dma_start(out=state0[:, NG:G, :], in_=d_x0t[:, NG:G, :])
            m01_sb = singles.tile([128, G, N], f16, tag="m01")
            nc.scalar.dma_start(out=m01_sb[:, 0:NG, :],
                                in_=d_m01[:, 0:NG * N].rearrange(
                                    "q (g p) -> q g p", g=NG))
            nc.sync.dma_start(out=m01_sb[:, NG:G, :],
                              in_=d_m01[:, NG * N:].rearrange(
                                  "q (g p) -> q g p", g=NG))
            b32_sb = singles.tile([128, 9], f32, tag="b32")
            nc.scalar.dma_start(out=b32_sb, in_=d_b32[:])
            cellT_sb = singles.tile([128, 4, G], f16, tag="cellT")
            nc.sync.dma_start(out=cellT_sb, in_=d_cellT[:])
            wB_sb = singles.tile([128, FB], f16, tag="wB")
            nc.scalar.dma_start(out=wB_sb, in_=d_wB[:])

            # weight views inside the blobs
            o = 0
            wg = wA_sb[0:ATOM + 1, o:o + 3 * ATOM].rearrange(
                "p (r d) -> p r d", r=3); o += 3 * ATOM
            wg1 = wA_sb[0:ATOM + 1, o:o + 3 * ATOM].rearrange(
                "p (r d) -> p r d", r=3); o += 3 * ATOM
            wg2 = wA_sb[0:ATOM + 1, o:o + 3 * ATOM].rearrange(
                "p (r d) -> p r d", r=3); o += 3 * ATOM
            s1c = wA_sb[0:ATOM, o:o + 3 * 128].rearrange(
                "p (r n) -> p r n", r=3); o += 3 * 128
            s2c = wA_sb[0:ATOM, o:o + 3 * 128].rearrange(
                "p (r n) -> p r n", r=3); o += 3 * 128
            i128 = wA_sb[:, o:o + 128]
            o = 0
            wtaug = wB_sb[0:ATOM + 1, o:o + 256].rearrange(
                "p (h m) -> p h m", h=2); o += 256
            wf = wB_sb[:, o:o + 68].rearrange("p (k d) -> p k d", k=2); o += 68
            wf2 = wB_sb[0:ATOM, o:o + 128]; o += 128
            w1 = wB_sb[:, o:o + 640].rearrange("p (k m) -> p k m", k=5); o += 640
            w2 = wB_sb[:, o:o + 256].rearrange("p (b m) -> p b m", b=2); o += 256
            w3 = wB_sb[:, o:o + 1024].rearrange(
                "p (k b m) -> p k b m", k=2, b=4); o += 1024
            w4 = wB_sb[:, o:o + 4]; o += 4
            b2n = b32_sb[:, 0:1]
            b1p = b32_sb[:, 1:2]
            b2d = b32_sb[:, 2:4]
            b3p = b32_sb[:, 4:8]
            b4p = b32_sb[0:1, 8:9]

            # ---------------- static SBUF tiles ---------------------------
            # ACT table warm-up (Exp/Relu/Prelu/Copy share one table set)
            warm = singles.tile([1, 1], f32, tag="warm")
            nc.vector.memset(warm, 0.0)
            nc.scalar.activation(out=warm, in_=warm, func=AF.Exp)

            haug = singles.tile([128, G, ATOM + 1], f16, tag="haug")
            nc.vector.memset(haug[:, :, ATOM], 1.0)
            ht1_sb = singles.tile([ATOM, G, N], f16, tag="ht1_sb")
            ht2_sb = singles.tile([ATOM, G, N], f16, tag="ht2_sb")
            Pu_sb = singles.tile([128, G, N], f16, tag="Pu_sb")
            P_sb = singles.tile([128, G, N], f16, tag="P_sb")
            inv = singles.tile([128, G], f16, tag="inv")
            dlt = singles.tile([128, G, ATOM], f16, tag="dlt")
            h1c_sb = singles.tile([128, G], f32, tag="h1c_sb")
            states = [state0]
            for r in range(3):
                st = singles.tile([ATOM + 1, G, N], f16, tag=f"state{r + 1}")
                states.append(st)
            # shared delta^T PSUM tile; row 34 stays 0 so the state update
            # can add all 35 rows (ones row survives: 1 + 0)
            dT_ps = pdT.tile([ATOM + 1, G, N], f16, tag="dT")
            nc.vector.memset(dT_ps.bitcast(f32), 0.0)

            # PE warm-up: dummy matmuls (no DMA deps) ramp the p-state
            zz = singles.tile([1, 128], f16, tag="zz")
            nc.vector.memset(zz, 0.0)
            misc_ps = pmisc.tile([128, 408], f32, tag="misc")
            hu_all = misc_ps[:, 0:280].rearrange("p (g c) -> p g c", g=NG)
            deco_ps = misc_ps[:, 280:408]
            wm_ps = deco_ps[0:1, 0:128]
            for i in range(N_WARM_MM):
                nc.tensor.matmul(wm_ps, zz[0:1, 0:1], zz, start=True, stop=True)

            # ---------------- GNN rounds ----------------------------------
            for r in range(3):
                S = states[r]
                Snx = states[r + 1]
                for gi, sl in groups:
                    ga = gi * NG
                    # ht1/ht2 = relu(|a|-scaled Wg^T @ state); then
                    # f_dst[q] = sum_d sign(a2)[d] ht2[d,q] etc. so the
                    # e matmul needs no materialized f vectors at all.
                    ht1_ps = pbig.tile([ATOM, NG, N], f32, tag="ht1")
                    nc.tensor.matmul(ht1_ps, wg1[:, r, :], S[:, sl, :],
                                     start=True, stop=True)
                    ht2_ps = pbig.tile([ATOM, NG, N], f32, tag="ht2")
                    nc.tensor.matmul(ht2_ps, wg2[:, r, :], S[:, sl, :],
                                     start=True, stop=True)
                    # h node-major per graph
                    hu_ps = hu_all
                    h_ps = hu_ps[:, :, 0:ATOM]
                    for g in range(NG):
                        nc.tensor.matmul(h_ps[:, g, :], S[:, ga + g, :],
                                         wg[:, r, 0:ATOM], start=True, stop=True)
                    nc.vector.tensor_scalar_max(ht1_sb[:, sl, :], ht1_ps, 0.0)
                    nc.vector.tensor_scalar_max(ht2_sb[:, sl, :], ht2_ps, 0.0)
                    nc.scalar.activation(out=haug[:, sl, 0:ATOM], in_=h_ps,
                                         func=AF.Relu)
                    # e[q,p] = f_dst[q] + f_src[p] via two accumulating mms
                    e_ps = patt.tile([128, NG, N], f32, tag="e", bufs=2)
                    for g in range(NG):
                        nc.tensor.matmul(e_ps[:, g, :], ht2_sb[:, ga + g, :],
                                         s2c[:, r, :], start=True, stop=False)
                        nc.tensor.matmul(e_ps[:, g, :], s1c[:, r, :],
                                         ht1_sb[:, ga + g, :],
                                         start=False, stop=True)
                    # t = lrelu(e)  (ACT Prelu, in-place on PSUM)
                    nc.scalar.activation(out=e_ps, in_=e_ps, func=AF.Prelu,
                                         alpha=0.01)
                    # Pu = exp(t) -> fp16 SBUF; mask multiply on Pool
                    nc.scalar.activation(out=Pu_sb[:, sl, :], in_=e_ps,
                                         func=AF.Exp)
                    nc.gpsimd.tensor_tensor(P_sb[:, sl, :], Pu_sb[:, sl, :],
                                            m01_sb[:, sl, :], OP.mult)
                    # U = P^T @ [h|1] : messages + row-sums
                    u_ps = hu_ps[:, :, ATOM:2 * ATOM + 1]
                    for g in range(NG):
                        nc.tensor.matmul(u_ps[:, g, :], P_sb[:, ga + g, :],
                                         haug[:, ga + g, :], start=True, stop=True)
                    nc.vector.reciprocal(inv[:, sl], u_ps[:, :, ATOM])
                    i_b = inv[:, sl].unsqueeze(2).to_broadcast([128, NG, ATOM])
                    nc.vector.tensor_tensor(dlt[:, sl, :], u_ps[:, :, 0:ATOM],
                                            i_b, OP.mult)
                    # delta^T per graph, then state update
                    for g in range(NG):
                        nc.tensor.transpose(dT_ps[0:ATOM, ga + g, :],
                                            dlt[:, ga + g, :], i128)
                    nc.vector.tensor_tensor(Snx[:, sl, :], S[:, sl, :],
                                            dT_ps[:, sl, :], OP.add)
                if dbg == r + 1:
                    xd = work.tile([ATOM, G, N], f32, tag="xd")
                    nc.vector.tensor_copy(xd, states[r + 1][0:ATOM])
                    nc.sync.dma_start(out=d_xdbg[:], in_=xd)
                if dbg in (4, 5, 6, 7, 8) and r == 0:
                    src = {4: ht1_sb, 5: P_sb, 6: ht2_sb,
                           7: haug, 8: dlt}[dbg]
                    xd = work.tile(_dbgshape, f32, tag="xd")
                    nc.vector.tensor_copy(xd, src)
                    nc.sync.dma_start(out=d_xdbg[:], in_=xd)

                # cell-branch of the decoder during round 0 (idle slots)
                if r == 0:
                    vc = singles.tile([128, 4, G], f16, tag="vc")
                    nc.scalar.activation(out=vc, in_=cellT_sb, func=AF.Exp,
                                         scale=-1.0)
                    nc.vector.tensor_scalar_add(vc, vc, 1.0)
                    nc.vector.reciprocal(vc, vc)
                    h1c_ps = deco_ps[:, 0:G]
                    for c in range(4):
                        nc.tensor.matmul(h1c_ps, w1[:, c + 1, :], vc[:, c, :],
                                         start=(c == 0), stop=(c == 3))
                    nc.vector.tensor_copy(h1c_sb, h1c_ps)

            # ---------------- VEC head + decoder --------------------------
            S3 = states[3]
            gts = singles.tile([128, 2, G, N], f16, tag="gts")
            d1_sb = singles.tile([ATOM, G, N], f16, tag="d1_sb")
            dm = singles.tile([128, G], f32, tag="dm")
            v0 = singles.tile([128, G], f16, tag="v0")
            h1 = singles.tile([128, G], f16, tag="h1")
            h2 = singles.tile([128, 2, G], f16, tag="h2")
            h3 = singles.tile([128, 4, G], f16, tag="h3")
            s_sb = singles.tile([1, G], f32, tag="s_sb")
            for gi, sl in groups:
                # g = relu(Wt^T @ state3), two 128-halves
                for hh in range(2):
                    gt_ps = pbig.tile([128, NG, N], f32, tag="gt")
                    nc.tensor.matmul(gt_ps, wtaug[:, hh, :], S3[:, sl, :],
                                     start=True, stop=True)
                    if hh == 0:
                        nc.scalar.activation(out=gts[:, hh, sl, :], in_=gt_ps,
                                             func=AF.Relu)
                    else:
                        nc.vector.tensor_scalar_max(gts[:, hh, sl, :], gt_ps, 0.0)
                # d1 = Wf^T @ g + x0   (residual via identity matmul)
                d1_ps = pdT.tile([ATOM, NG, N], f32, tag="d1")
                for k in range(2):
                    nc.tensor.matmul(d1_ps, wf[:, k, :], gts[:, k, sl, :],
                                     start=(k == 0), stop=False)
                nc.tensor.matmul(d1_ps, i128[0:ATOM, 0:ATOM],
                                 state0[0:ATOM, sl, :], start=False, stop=True)
                nc.vector.tensor_scalar_max(d1_sb[:, sl, :], d1_ps, -60000.0)
                # d2 = Wf2^T @ d1 ; dm = max over nodes
                d2_ps = pbig.tile([128, NG, N], f32, tag="gt")
                nc.tensor.matmul(d2_ps, wf2, d1_sb[:, sl, :],
                                 start=True, stop=True)
                nc.vector.tensor_reduce(dm[:, sl], d2_ps, AX.X, OP.max)
                # v0 = sigmoid(dm + bias) = 1/(1+exp(-dm+b2n))
                nc.scalar.activation(out=v0[:, sl], in_=dm[:, sl], func=AF.Exp,
                                     bias=b2n, scale=-1.0)
                nc.vector.tensor_scalar_add(v0[:, sl], v0[:, sl], 1.0)
                nc.vector.reciprocal(v0[:, sl], v0[:, sl])

                # ---- decoder for this group (overlaps the other group) ----
                c0 = 8 + 4 * gi
                h1_psg = deco_ps[:, c0:c0 + NG]
                nc.tensor.matmul(h1_psg, w1[:, 0, :], v0[:, sl],
                                 start=True, stop=True)
                nc.vector.scalar_tensor_tensor(h1[:, sl], h1_psg, b1p,
                                               h1c_sb[:, sl], OP.add, OP.add)
                nc.vector.tensor_scalar_max(h1[:, sl], h1[:, sl], 0.0)
                c0 = 16 + 8 * gi
                h2_psg = deco_ps[:, c0:c0 + 8].rearrange(
                    "p (b g) -> p b g", b=2)
                for b in range(2):
                    nc.tensor.matmul(h2_psg[:, b, :], w2[:, b, :], h1[:, sl],
                                     start=True, stop=True)
                b2d_b = b2d.unsqueeze(2).to_broadcast([128, 2, NG])
                nc.vector.scalar_tensor_tensor(h2[:, :, sl], h2_psg, 1.0,
                                               b2d_b, OP.mult, OP.add)
                nc.vector.tensor_scalar_max(h2[:, :, sl], h2[:, :, sl], 0.0)
                c0 = 32 + 16 * gi
                h3_psg = deco_ps[:, c0:c0 + 16].rearrange(
                    "p (b g) -> p b g", b=4)
                for b in range(4):
                    for kc in range(2):
                        nc.tensor.matmul(h3_psg[:, b, :], w3[:, kc, b, :],
                                         h2[:, kc, sl], start=(kc == 0),
                                         stop=(kc == 1))
                b3_b = b3p.unsqueeze(2).to_broadcast([128, 4, NG])
                nc.vector.scalar_tensor_tensor(h3[:, :, sl], h3_psg, 1.0,
                                               b3_b, OP.mult, OP.add)
                nc.vector.tensor_scalar_max(h3[:, :, sl], h3[:, :, sl], 0.0)
                c0 = 64 + 4 * gi
                s_psg = deco_ps[0:1, c0:c0 + NG]
                for c in range(4):
                    nc.tensor.matmul(s_psg, w4[:, c:c + 1], h3[:, c, sl],
                                     start=(c == 0), stop=(c == 3))
                nc.vector.tensor_scalar_add(s_sb[:, sl], s_psg, b4p)
            if dbg == 9:
                xd = work.tile([128, 2, G, N], f32, tag="xd")
                nc.vector.tensor_copy(xd, gts)
                nc.sync.dma_start(out=d_xdbg[:], in_=xd)
            if dbg == 10:
                xd = work.tile([ATOM, G, N], f32, tag="xd")
                nc.vector.tensor_copy(xd, d1_sb)
                nc.sync.dma_start(out=d_xdbg[:], in_=xd)
            if dbg == 11:
                nc.sync.dma_start(out=d_xdbg[:], in_=dm)
            if dbg == 12:
                xd = work.tile([128, G], f32, tag="xd")
                nc.vector.tensor_copy(xd, v0)
                nc.sync.dma_start(out=d_xdbg[:], in_=xd)

            if dbg == 15:
                nc.sync.dma_start(out=d_xdbg[:], in_=s_sb)
            nc.sync.dma_start(out=d_score.rearrange("g x -> x g"), in_=s_sb)

    lowp.__exit__(None, None, None)
    return nc


def _fix_preamble_regs(nc):
    """Bacc defers register allocation; its alloc_regs pass skips the
    framework preamble registers, leaving reg_id=-1 which walrus rejects.
    Assign collision-free ids."""
    per_engine_used = {}
    pending = []
    for alloc in nc.m.functions[0].allocations:
        eng = getattr(alloc, "engine", None)
        rid = getattr(alloc, "reg_id", None)
        if eng is None or rid is None:
            continue
        if rid >= 0:
            per_engine_used.setdefault(eng, set()).add(rid)
        else:
            pending.append(alloc)
    canonical = {"zero": 8, "monotonic_0_cnt": 9, "bcreg0_lo": 10,
                 "bcreg0_hi": 11, "bcreg1_lo": 12, "bcreg1_hi": 13,
                 "tpb_base_lo": 14, "tpb_base_hi": 15}
    for alloc in pending:
        eng = alloc.engine
        used = per_engine_used.setdefault(eng, set())
        suffix = alloc.name.split("_", 1)[1] if "_" in alloc.name else alloc.name
        rid = canonical.get(suffix, 16)
        while rid in used:
            rid += 1
        alloc.reg_id = rid
        used.add(rid)


def _stage(inputs):
    """Host-side layout staging (fp16/fp8 packing). Returns per-core in_maps."""
    import ml_dtypes

    f16 = np.float16
    xs = np.asarray(inputs["xs"], dtype=np.float32)
    A = np.asarray(inputs["A"])
    cell = np.asarray(inputs["cell_emb"], dtype=np.float32)
    Wg = np.asarray(inputs["Wg"], dtype=np.float32)
    bg = np.asarray(inputs["bg"], dtype=np.float32)
    attn = np.asarray(inputs["attn"], dtype=np.float32)
    Wt = np.asarray(inputs["Wt"], dtype=np.float32)
    bt = np.asarray(inputs["bt"], dtype=np.float32)
    Wf = np.asarray(inputs["Wf"], dtype=np.float32)
    bf = np.asarray(inputs["bf"], dtype=np.float32)
    Wf2 = np.asarray(inputs["Wf2"], dtype=np.float32)
    bf2 = np.asarray(inputs["bf2"], dtype=np.float32)
    W1 = np.asarray(inputs["W1"], dtype=np.float32)
    b1 = np.asarray(inputs["b1"], dtype=np.float32)
    W2 = np.asarray(inputs["W2"], dtype=np.float32)
    b2 = np.asarray(inputs["b2"], dtype=np.float32)
    W3 = np.asarray(inputs["W3"], dtype=np.float32)
    b3 = np.asarray(inputs["b3"], dtype=np.float32)
    W4 = np.asarray(inputs["W4"], dtype=np.float32)
    b4 = np.asarray(inputs["b4"], dtype=np.float32)

    FA = 9 * ATOM + 6 * 128 + 128
    wA = np.zeros((128, FA), np.float32)
    a1 = attn[:, :ATOM]
    a2 = attn[:, ATOM:]
    wg4 = np.zeros((ATOM + 1, 3, ATOM), np.float32)
    wg4[:ATOM] = Wg.transpose(1, 0, 2)
    wg4[ATOM] = bg
    o = 0
    wA[:ATOM + 1, o:o + 102] = wg4.reshape(ATOM + 1, 102); o += 102
    wgs1 = wg4 * np.abs(a1).T[None, :, :].transpose(0, 1, 2)
    wgs1 = wg4 * np.abs(a1)[None, :, :]
    wA[:ATOM + 1, o:o + 102] = wgs1.reshape(ATOM + 1, 102); o += 102
    wgs2 = wg4 * np.abs(a2)[None, :, :]
    wA[:ATOM + 1, o:o + 102] = wgs2.reshape(ATOM + 1, 102); o += 102
    s1c = np.repeat(np.sign(a1).T[:, :, None], 128, axis=2)  # [34, 3, 128]
    wA[:ATOM, o:o + 384] = s1c.transpose(0, 1, 2).reshape(ATOM, 384); o += 384
    s2c = np.repeat(np.sign(a2).T[:, :, None], 128, axis=2)
    wA[:ATOM, o:o + 384] = s2c.reshape(ATOM, 384); o += 384
    wA[:, o:o + 128] = np.eye(128, dtype=np.float32); o += 128

    FB = 256 + 68 + 128 + 640 + 256 + 1024 + 4
    wB = np.zeros((128, FB), np.float32)
    o = 0
    wB[:ATOM, o:o + 256] = Wt
    wB[ATOM, o:o + 256] = bt
    o += 256
    wB[:, o:o + 68] = Wf.reshape(2, 128, ATOM).transpose(1, 0, 2).reshape(128, 68)
    o += 68
    wB[:ATOM, o:o + 128] = Wf2
    o += 128
    wB[:, o:o + 640] = W1.reshape(5, 128, 128).transpose(1, 0, 2).reshape(128, 640)
    o += 640
    wB[:, o:o + 256] = W2.reshape(128, 2, 128).reshape(128, 256)
    o += 256
    wB[:, o:o + 1024] = W3.reshape(2, 128, 4, 128).transpose(1, 0, 2, 3).reshape(128, 1024)
    o += 1024
    wB[:, o:o + 4] = W4.reshape(4, 128).T
    o += 4

    b32 = np.zeros((128, 9), np.float32)
    b32[:, 0] = -(bf @ Wf2 + bf2)
    b32[:, 1] = b1
    b32[:, 2:4] = b2.reshape(2, 128).T
    b32[:, 4:8] = b3.reshape(4, 128).T
    b32[0, 8] = b4[0]

    shared = dict(wA=wA.astype(f16), wB=wB.astype(f16), b32=b32)

    in_maps = []
    for core in range(NCORES):
        sl = slice(core * G, (core + 1) * G)
        x0t = np.concatenate(
            [xs[sl].transpose(0, 2, 1),
             np.ones((G, 1, N), np.float32)], axis=1)      # [G, 35, N]
        x0t = np.ascontiguousarray(x0t.transpose(1, 0, 2)).astype(f16)
        m_qgp = (np.asarray(A[sl]) > 0).transpose(2, 0, 1)  # [q, g, p]
        m01 = m_qgp.astype(f16).reshape(128, G * N)
        cellT = np.ascontiguousarray(
            cell[sl].reshape(G, 4, 128).transpose(2, 1, 0)).astype(f16)
        m = dict(shared)
        m.update(x0t=x0t, m01=m01, cellT=cellT)
        in_maps.append(m)
    return in_maps


def get_nc(dbg=0):
    key = f"nc{dbg}"
    if key not in _CACHE:
        nc = _build_bass(dbg)
        nc.finalize()
        _fix_preamble_regs(nc)
        _CACHE[key] = nc
    return _CACHE[key]


def kernel(**inputs) -> np.ndarray:
    from concourse.bass_utils import run_bass_kernel_spmd

    nc = get_nc()
    in_maps = _stage(inputs)
    res = run_bass_kernel_spmd(nc, in_maps, core_ids=list(range(NCORES)))
    out = np.concatenate([res.results[i]["score"] for i in range(NCORES)], axis=0)
    return out.astype(np.float32)
